# revision 40
# baseline (speedup 1.0000x reference)
"""HadamardTest kernel for Trainium2, 8-core data-parallel SPMD.

out[n, c] = (z_re @ refT)^2 + (z_im @ refT)^2, ref = L2-normalized zero-padded canon.

Sharding: z_re/z_im split along samples into 8 shards of 2048 rows; the tiny
normalized 1024x10 refT table is replicated. Each core computes its
[10, 2048] slice of the (transposed) output.

Device algorithm per core (fp32r end-to-end, no convert pass):
  - z loaded in natural [sample, dim] layout (contiguous 4KB-row DMA, split
    into half-block pieces across the SP and GpSimd DGE queues) into fp32r
    tiles,
  - PE transpose (exact, via fp32r identity) gives [dim, sample] tiles in PSUM,
  - PSUM -> SBUF copies alternate between DVE and Activation,
  - fp32r matmuls with the refT chunk stationary accumulate <z|ref> in PSUM,
  - epilogue: Act squares s_re and s_im (PSUM->SBUF fused), DVE adds;
    result DMAed out as [10, 2048].

kernel() caches the compiled SPMD executable after the first call; every call
still transfers the full inputs, executes on all 8 cores, and fetches the
full output.
"""
import numpy as np

import concourse.mybir as mybir
import concourse.tile as tile
from concourse import bacc
from concourse._compat import axon_active

F32 = mybir.dt.float32
F32R = mybir.dt.float32r
AF = mybir.ActivationFunctionType

N = 16384          # total samples
DIM = 1024         # state dimension (2**10)
C = 10             # classes
IMG = 784          # 28*28 pixels before zero-pad
N_CORES = 8
NS = N // N_CORES  # 2048 samples per core
NT = 512           # sample-block (matmul moving free dim)
NB = NS // NT      # 4 blocks per core
KCH = DIM // 128   # 8 contraction chunks
P = 128

_CACHE = {}
_HALF_SPLIT = False  # split each block-part load into two half DMAs
_GPSIMD_HALVES = True  # route later blocks' second halves via gpsimd DGE


def build_kernel(repeat=None):
    key = ("nc", repeat)
    if key in _CACHE:
        return _CACHE[key]
    nc = bacc.Bacc(None, target_bir_lowering=False, debug=False,
                   num_devices=N_CORES)
    zre_d = nc.dram_tensor("z_re", [NS, DIM], F32R, kind="ExternalInput").ap()
    zim_d = nc.dram_tensor("z_im", [NS, DIM], F32R, kind="ExternalInput").ap()
    refT_d = nc.dram_tensor("refT", [DIM, C], F32R, kind="ExternalInput").ap()
    ident_d = nc.dram_tensor("ident", [P, P], F32R, kind="ExternalInput").ap()
    outT_d = nc.dram_tensor("outT", [C, NS], F32, kind="ExternalOutput").ap()

    with tile.TileContext(nc) as tc:
        with (
            tc.tile_pool(name="const", bufs=1) as cpool,
            tc.tile_pool(name="zload", bufs=4) as zpool,
            tc.tile_pool(name="ztr", bufs=8) as ztpool,
            tc.tile_pool(name="outsb", bufs=3) as opool,
            tc.tile_pool(name="tpsum", bufs=5, space="PSUM") as tpsum,
            tc.tile_pool(name="opsum", bufs=3, space="PSUM") as opsum,
        ):
            rt = cpool.tile([P, KCH, C], F32R)
            nc.sync.dma_start(
                out=rt[:], in_=refT_d.rearrange("(k p) c -> p k c", p=P))
            idt = cpool.tile([P, P], F32R)
            nc.sync.dma_start(out=idt[:], in_=ident_d[:])

            # PSUM->SBUF copy, alternating between the two PSUM-capable
            # non-PE engines (GPSIMD/Pool cannot access PSUM on TRN2)
            def stage_copy(idx, zt, st):
                if idx % 2 == 0:
                    nc.vector.tensor_copy(zt[:], st[:])
                else:
                    nc.scalar.activation(out=zt[:], in_=st[:], func=AF.Copy)

            # repeat=R unrolls the whole body R times (device-time measurement)
            for _rep in range(repeat or 1):
              for nt in range(NB):
                  ps_out = []
                  o = opool.tile([C, NT], F32, tag="o")
                  t2 = opool.tile([C, NT], F32, tag="t2")
                  for pi, zd in enumerate((zre_d, zim_d)):
                      znat = zpool.tile([P, NT // P, DIM], F32R, tag="znat")
                      if _HALF_SPLIT:
                          # two DMAs per block-part (halves): finer deps
                          # unblock transposes sooner. The second-half loads
                          # of the later blocks go through gpsimd's DGE queue
                          # (they are gated on znat buffer recycling anyway),
                          # keeping SP sequencer config off the critical path.
                          half = NT // 2
                          eng1 = (nc.gpsimd if _GPSIMD_HALVES and nt >= NB // 2
                                  else nc.sync)
                          for h, eng in ((0, nc.sync), (1, eng1)):
                              eng.dma_start(
                                  out=znat[:, h * 2:(h + 1) * 2],
                                  in_=zd[nt * NT + h * half:
                                         nt * NT + (h + 1) * half]
                                      .rearrange("(j p) d -> p j d", p=P))
                      else:
                          nc.sync.dma_start(
                              out=znat[:],
                              in_=zd[nt * NT:(nt + 1) * NT]
                                  .rearrange("(j p) d -> p j d", p=P))
                      ps_o = opsum.tile([C, NT], F32, tag="po")
                      ps_out.append(ps_o)
                      for dk in range(KCH):
                          st = tpsum.tile([P, NT], F32R, tag="tstage")
                          for j in range(NT // P):
                              nc.tensor.transpose(
                                  st[:, j * P:(j + 1) * P],
                                  znat[:, j, dk * P:(dk + 1) * P],
                                  idt[:])
                          zt = ztpool.tile([P, NT], F32R, tag="zt")
                          stage_copy((nt * 2 + pi) * KCH + dk, zt, st)
                          nc.tensor.matmul(
                              ps_o[:], rt[:, dk], zt[:],
                              start=(dk == 0), stop=(dk == KCH - 1))
                      if pi == 0:
                          # square s_re as soon as its accumulation is done,
                          # overlapping the im chunk processing instead of
                          # serializing in the block's epilogue
                          nc.scalar.activation(out=o[:], in_=ps_o[:],
                                               func=AF.Square)
                  nc.scalar.activation(out=t2[:], in_=ps_out[1][:], func=AF.Square)
                  nc.vector.tensor_add(out=o[:], in0=o[:], in1=t2[:])
                  nc.sync.dma_start(out=outT_d[:, nt * NT:(nt + 1) * NT], in_=o[:])

    nc.finalize()
    _CACHE[key] = nc
    return nc


def _prep_host(z_re, z_im, canon):
    ref = np.asarray(canon, dtype=np.float32).reshape(C, IMG)
    ref = np.pad(ref, ((0, 0), (0, DIM - IMG)))
    ref = ref / np.linalg.norm(ref, axis=1, keepdims=True)
    refT = np.ascontiguousarray(ref.T)                      # [DIM, C]
    ident = np.eye(P, dtype=np.float32)
    z_re = np.ascontiguousarray(np.asarray(z_re, dtype=np.float32))
    z_im = np.ascontiguousarray(np.asarray(z_im, dtype=np.float32))
    return z_re, z_im, refT, ident


def prepare_in_maps(z_re, z_im, canon):
    z_re, z_im, refT, ident = _prep_host(z_re, z_im, canon)
    return [
        {
            "z_re": z_re[c * NS:(c + 1) * NS],
            "z_im": z_im[c * NS:(c + 1) * NS],
            "refT": refT,
            "ident": ident,
        }
        for c in range(N_CORES)
    ]


class _Runner:
    """Cached compiled SPMD callable over device-sharded inputs (axon/PJRT).

    Mirrors concourse.bass2jax.run_bass_via_pjrt's lowering, but compiles
    once (fast-dispatch, no donation) and is reused across kernel() calls.
    """

    def __init__(self, nc):
        import jax
        from jax.sharding import Mesh, PartitionSpec, NamedSharding
        from jax.experimental.shard_map import shard_map
        from concourse.bass2jax import (
            _bass_exec_p, fast_dispatch_compile, install_neuronx_cc_hook,
            partition_id_tensor,
        )

        install_neuronx_cc_hook()
        self.jax = jax
        partition_name = (nc.partition_id_tensor.name
                          if nc.partition_id_tensor else None)
        in_names, out_names, out_avals, zero_outs = [], [], [], []
        for alloc in nc.m.functions[0].allocations:
            if not isinstance(alloc, mybir.MemoryLocationSet):
                continue
            name = alloc.memorylocations[0].name
            if alloc.kind == "ExternalInput":
                if name != partition_name:
                    in_names.append(name)
            elif alloc.kind == "ExternalOutput":
                shape = tuple(alloc.tensor_shape)
                dtype = mybir.dt.np(alloc.dtype)
                out_names.append(name)
                out_avals.append(jax.core.ShapedArray(shape, dtype))
                zero_outs.append(np.zeros(shape, dtype))
        self.in_names, self.out_names = in_names, out_names
        self.out_avals = out_avals
        all_in = list(in_names) + list(out_names)
        if partition_name is not None:
            all_in.append(partition_name)

        def _body(*args):
            operands = list(args)
            if partition_name is not None:
                operands.append(partition_id_tensor())
            return tuple(_bass_exec_p.bind(
                *operands,
                out_avals=tuple(out_avals),
                in_names=tuple(all_in),
                out_names=tuple(out_names),
                lowering_input_output_aliases=(),
                sim_require_finite=True,
                sim_require_nnan=True,
                nc=nc,
            ))

        devices = jax.devices()[:N_CORES]
        mesh = Mesh(np.asarray(devices), ("core",))
        spec = PartitionSpec("core")
        self.sharding = NamedSharding(mesh, spec)
        n_args = len(in_names) + len(out_names)
        self.zero_concat = [
            jax.device_put(
                np.zeros((N_CORES * z.shape[0], *z.shape[1:]), z.dtype),
                self.sharding)
            for z in zero_outs
        ]
        example = tuple(
            jax.ShapeDtypeStruct((N_CORES * a, *rest), dt, sharding=self.sharding)
            for (a, *rest), dt in [
                ((NS, DIM), np.float32), ((NS, DIM), np.float32),
                ((DIM, C), np.float32), ((P, P), np.float32),
            ]
        ) + tuple(
            jax.ShapeDtypeStruct(z.shape, z.dtype, sharding=self.sharding)
            for z in self.zero_concat
        )
        assert len(example) == n_args

        def _compile():
            return (
                jax.jit(
                    shard_map(_body, mesh=mesh, in_specs=(spec,) * n_args,
                              out_specs=(spec,) * len(out_names),
                              check_rep=False),
                    keep_unused=True,
                )
                .lower(*example)
                .compile()
            )

        self.fn = fast_dispatch_compile(_compile)

    def __call__(self, z_re, z_im, refT, ident):
        jax = self.jax
        put = lambda x: jax.device_put(x, self.sharding)
        args = (
            put(z_re),                                   # [N, DIM] -> shards
            put(z_im),
            put(np.broadcast_to(refT, (N_CORES,) + refT.shape)
                .reshape(N_CORES * DIM, C)),
            put(np.broadcast_to(ident, (N_CORES,) + ident.shape)
                .reshape(N_CORES * P, P)),
        ) + tuple(self.zero_concat)
        outs = self.fn(*args)
        outT = np.asarray(outs[0]).reshape(N_CORES, C, NS)   # [8, 10, 2048]
        return np.ascontiguousarray(
            outT.transpose(0, 2, 1).reshape(N, C))


_RUNNER = None


def kernel(z_re, z_im, canon):
    z_re, z_im, refT, ident = _prep_host(z_re, z_im, canon)
    if axon_active():
        global _RUNNER
        if _RUNNER is None:
            _RUNNER = _Runner(build_kernel())
        return _RUNNER(z_re, z_im, refT, ident)
    # native (non-axon) fallback: one-shot run via the stock SPMD driver
    from concourse.bass_utils import run_bass_kernel_spmd
    nc = build_kernel()
    in_maps = prepare_in_maps(z_re, z_im, canon)
    res = run_bass_kernel_spmd(nc, in_maps, list(range(N_CORES)), trace=False)
    out = np.empty((N, C), dtype=np.float32)
    for c in range(N_CORES):
        out[c * NS:(c + 1) * NS] = res.results[c]["outT"].T
    return out


# revision 43
# speedup vs baseline: 1.1219x; 1.1219x over previous
"""HadamardTest kernel for Trainium2, 8-core data-parallel SPMD.

out[n, c] = (z_re @ refT)^2 + (z_im @ refT)^2, ref = L2-normalized zero-padded canon.

Sharding: z_re/z_im split along samples into 8 shards of 2048 rows; the tiny
normalized 1024x10 refT table is replicated. Each core computes its
[10, 2048] slice of the (transposed) output.

Device algorithm per core (fp32r end-to-end, no convert pass):
  - z loaded in natural [sample, dim] layout (contiguous 4KB-row DMA, split
    into half-block pieces across the SP and GpSimd DGE queues) into fp32r
    tiles,
  - PE transpose (exact, via fp32r identity) gives [dim, sample] tiles in PSUM,
  - PSUM -> SBUF copies alternate between DVE and Activation,
  - fp32r matmuls with the refT chunk stationary accumulate <z|ref> in PSUM,
  - epilogue: Act squares s_re and s_im (PSUM->SBUF fused), DVE adds;
    result DMAed out as [10, 2048].

kernel() caches the compiled SPMD executable after the first call; every call
still transfers the full inputs, executes on all 8 cores, and fetches the
full output.
"""
import numpy as np

import concourse.mybir as mybir
import concourse.tile as tile
from concourse import bacc
from concourse._compat import axon_active

F32 = mybir.dt.float32
F32R = mybir.dt.float32r
AF = mybir.ActivationFunctionType

N = 16384          # total samples
DIM = 1024         # state dimension (2**10)
C = 10             # classes
IMG = 784          # 28*28 pixels before zero-pad
N_CORES = 8
NS = N // N_CORES  # 2048 samples per core
NT = 512           # sample-block (matmul moving free dim)
NB = NS // NT      # 4 blocks per core
KCH = DIM // 128   # 8 contraction chunks
P = 128

_CACHE = {}
_HALF_SPLIT = False  # split each block-part load into two half DMAs
_GPSIMD_HALVES = True  # route later blocks' second halves via gpsimd DGE
_PJ_LAYOUT = False   # 16KB-contiguous per-partition loads (permutes outputs)


def build_kernel(repeat=None):
    key = ("nc", repeat)
    if key in _CACHE:
        return _CACHE[key]
    nc = bacc.Bacc(None, target_bir_lowering=False, debug=False,
                   num_devices=N_CORES)
    zre_d = nc.dram_tensor("z_re", [NS, DIM], F32R, kind="ExternalInput").ap()
    zim_d = nc.dram_tensor("z_im", [NS, DIM], F32R, kind="ExternalInput").ap()
    refT_d = nc.dram_tensor("refT", [DIM, C], F32R, kind="ExternalInput").ap()
    ident_d = nc.dram_tensor("ident", [P, P], F32R, kind="ExternalInput").ap()
    outT_d = nc.dram_tensor("outT", [C, NS], F32, kind="ExternalOutput").ap()

    with tile.TileContext(nc) as tc:
        with (
            tc.tile_pool(name="const", bufs=1) as cpool,
            tc.tile_pool(name="zload", bufs=6) as zpool,
            tc.tile_pool(name="ztr", bufs=8) as ztpool,
            tc.tile_pool(name="outsb", bufs=3) as opool,
            tc.tile_pool(name="tpsum", bufs=5, space="PSUM") as tpsum,
            tc.tile_pool(name="opsum", bufs=3, space="PSUM") as opsum,
        ):
            rt = cpool.tile([P, KCH, C], F32R)
            nc.sync.dma_start(
                out=rt[:], in_=refT_d.rearrange("(k p) c -> p k c", p=P))
            idt = cpool.tile([P, P], F32R)
            nc.sync.dma_start(out=idt[:], in_=ident_d[:])

            # PSUM->SBUF copy, alternating between the two PSUM-capable
            # non-PE engines (GPSIMD/Pool cannot access PSUM on TRN2)
            def stage_copy(idx, zt, st):
                if idx % 2 == 0:
                    nc.vector.tensor_copy(zt[:], st[:])
                else:
                    nc.scalar.activation(out=zt[:], in_=st[:], func=AF.Copy)

            # repeat=R unrolls the whole body R times (device-time measurement)
            for _rep in range(repeat or 1):
              for nt in range(NB):
                  ps_out = []
                  o = opool.tile([C, NT], F32, tag="o")
                  t2 = opool.tile([C, NT], F32, tag="t2")
                  for pi, zd in enumerate((zre_d, zim_d)):
                      znat = zpool.tile([P, NT // P, DIM], F32R, tag="znat")
                      if _HALF_SPLIT:
                          # two DMAs per block-part (halves): finer deps
                          # unblock transposes sooner. The second-half loads
                          # of the later blocks go through gpsimd's DGE queue
                          # (they are gated on znat buffer recycling anyway),
                          # keeping SP sequencer config off the critical path.
                          half = NT // 2
                          eng1 = (nc.gpsimd if _GPSIMD_HALVES and nt >= NB // 2
                                  else nc.sync)
                          for h, eng in ((0, nc.sync), (1, eng1)):
                              eng.dma_start(
                                  out=znat[:, h * 2:(h + 1) * 2],
                                  in_=zd[nt * NT + h * half:
                                         nt * NT + (h + 1) * half]
                                      .rearrange("(j p) d -> p j d", p=P))
                      elif _PJ_LAYOUT:
                          # partition p <- rows 4p..4p+3: one contiguous 16KB
                          # DRAM read per partition (vs 4x 4KB at 512KB
                          # stride). Output columns come out permuted as
                          # (j p) <-> sample 4p+j; undone on the host.
                          nc.sync.dma_start(
                              out=znat[:],
                              in_=zd[nt * NT:(nt + 1) * NT]
                                  .rearrange("(p j) d -> p j d", j=NT // P))
                      else:
                          nc.sync.dma_start(
                              out=znat[:],
                              in_=zd[nt * NT:(nt + 1) * NT]
                                  .rearrange("(j p) d -> p j d", p=P))
                      ps_o = opsum.tile([C, NT], F32, tag="po")
                      ps_out.append(ps_o)
                      for dk in range(KCH):
                          st = tpsum.tile([P, NT], F32R, tag="tstage")
                          for j in range(NT // P):
                              nc.tensor.transpose(
                                  st[:, j * P:(j + 1) * P],
                                  znat[:, j, dk * P:(dk + 1) * P],
                                  idt[:])
                          zt = ztpool.tile([P, NT], F32R, tag="zt")
                          stage_copy((nt * 2 + pi) * KCH + dk, zt, st)
                          nc.tensor.matmul(
                              ps_o[:], rt[:, dk], zt[:],
                              start=(dk == 0), stop=(dk == KCH - 1))
                      if pi == 0:
                          # square s_re as soon as its accumulation is done,
                          # overlapping the im chunk processing instead of
                          # serializing in the block's epilogue
                          nc.scalar.activation(out=o[:], in_=ps_o[:],
                                               func=AF.Square)
                  nc.scalar.activation(out=t2[:], in_=ps_out[1][:], func=AF.Square)
                  nc.vector.tensor_add(out=o[:], in0=o[:], in1=t2[:])
                  nc.sync.dma_start(out=outT_d[:, nt * NT:(nt + 1) * NT], in_=o[:])

    nc.finalize()
    _CACHE[key] = nc
    return nc


def _prep_host(z_re, z_im, canon):
    ref = np.asarray(canon, dtype=np.float32).reshape(C, IMG)
    ref = np.pad(ref, ((0, 0), (0, DIM - IMG)))
    ref = ref / np.linalg.norm(ref, axis=1, keepdims=True)
    refT = np.ascontiguousarray(ref.T)                      # [DIM, C]
    ident = np.eye(P, dtype=np.float32)
    z_re = np.ascontiguousarray(np.asarray(z_re, dtype=np.float32))
    z_im = np.ascontiguousarray(np.asarray(z_im, dtype=np.float32))
    return z_re, z_im, refT, ident


def prepare_in_maps(z_re, z_im, canon):
    z_re, z_im, refT, ident = _prep_host(z_re, z_im, canon)
    return [
        {
            "z_re": z_re[c * NS:(c + 1) * NS],
            "z_im": z_im[c * NS:(c + 1) * NS],
            "refT": refT,
            "ident": ident,
        }
        for c in range(N_CORES)
    ]


class _Runner:
    """Cached compiled SPMD callable over device-sharded inputs (axon/PJRT).

    Mirrors concourse.bass2jax.run_bass_via_pjrt's lowering, but compiles
    once (fast-dispatch, no donation) and is reused across kernel() calls.
    """

    def __init__(self, nc):
        import jax
        from jax.sharding import Mesh, PartitionSpec, NamedSharding
        from jax.experimental.shard_map import shard_map
        from concourse.bass2jax import (
            _bass_exec_p, fast_dispatch_compile, install_neuronx_cc_hook,
            partition_id_tensor,
        )

        install_neuronx_cc_hook()
        self.jax = jax
        partition_name = (nc.partition_id_tensor.name
                          if nc.partition_id_tensor else None)
        in_names, out_names, out_avals, zero_outs = [], [], [], []
        for alloc in nc.m.functions[0].allocations:
            if not isinstance(alloc, mybir.MemoryLocationSet):
                continue
            name = alloc.memorylocations[0].name
            if alloc.kind == "ExternalInput":
                if name != partition_name:
                    in_names.append(name)
            elif alloc.kind == "ExternalOutput":
                shape = tuple(alloc.tensor_shape)
                dtype = mybir.dt.np(alloc.dtype)
                out_names.append(name)
                out_avals.append(jax.core.ShapedArray(shape, dtype))
                zero_outs.append(np.zeros(shape, dtype))
        self.in_names, self.out_names = in_names, out_names
        self.out_avals = out_avals
        all_in = list(in_names) + list(out_names)
        if partition_name is not None:
            all_in.append(partition_name)

        def _body(*args):
            operands = list(args)
            if partition_name is not None:
                operands.append(partition_id_tensor())
            return tuple(_bass_exec_p.bind(
                *operands,
                out_avals=tuple(out_avals),
                in_names=tuple(all_in),
                out_names=tuple(out_names),
                lowering_input_output_aliases=(),
                sim_require_finite=True,
                sim_require_nnan=True,
                nc=nc,
            ))

        devices = jax.devices()[:N_CORES]
        mesh = Mesh(np.asarray(devices), ("core",))
        spec = PartitionSpec("core")
        self.sharding = NamedSharding(mesh, spec)
        n_args = len(in_names) + len(out_names)
        self.zero_concat = [
            jax.device_put(
                np.zeros((N_CORES * z.shape[0], *z.shape[1:]), z.dtype),
                self.sharding)
            for z in zero_outs
        ]
        example = tuple(
            jax.ShapeDtypeStruct((N_CORES * a, *rest), dt, sharding=self.sharding)
            for (a, *rest), dt in [
                ((NS, DIM), np.float32), ((NS, DIM), np.float32),
                ((DIM, C), np.float32), ((P, P), np.float32),
            ]
        ) + tuple(
            jax.ShapeDtypeStruct(z.shape, z.dtype, sharding=self.sharding)
            for z in self.zero_concat
        )
        assert len(example) == n_args

        def _compile():
            return (
                jax.jit(
                    shard_map(_body, mesh=mesh, in_specs=(spec,) * n_args,
                              out_specs=(spec,) * len(out_names),
                              check_rep=False),
                    keep_unused=True,
                )
                .lower(*example)
                .compile()
            )

        self.fn = fast_dispatch_compile(_compile)

    def __call__(self, z_re, z_im, refT, ident):
        jax = self.jax
        put = lambda x: jax.device_put(x, self.sharding)
        args = (
            put(z_re),                                   # [N, DIM] -> shards
            put(z_im),
            put(np.broadcast_to(refT, (N_CORES,) + refT.shape)
                .reshape(N_CORES * DIM, C)),
            put(np.broadcast_to(ident, (N_CORES,) + ident.shape)
                .reshape(N_CORES * P, P)),
        ) + tuple(self.zero_concat)
        outs = self.fn(*args)
        outT = np.asarray(outs[0]).reshape(N_CORES, C, NS)   # [8, 10, 2048]
        return np.ascontiguousarray(
            outT.transpose(0, 2, 1).reshape(N, C))


_RUNNER = None


def kernel(z_re, z_im, canon):
    z_re, z_im, refT, ident = _prep_host(z_re, z_im, canon)
    if axon_active():
        global _RUNNER
        if _RUNNER is None:
            _RUNNER = _Runner(build_kernel())
        return _RUNNER(z_re, z_im, refT, ident)
    # native (non-axon) fallback: one-shot run via the stock SPMD driver
    from concourse.bass_utils import run_bass_kernel_spmd
    nc = build_kernel()
    in_maps = prepare_in_maps(z_re, z_im, canon)
    res = run_bass_kernel_spmd(nc, in_maps, list(range(N_CORES)), trace=False)
    out = np.empty((N, C), dtype=np.float32)
    for c in range(N_CORES):
        out[c * NS:(c + 1) * NS] = res.results[c]["outT"].T
    return out
